# revision 17
# baseline (speedup 1.0000x reference)
"""AcceptRejectPooling2D on 8 Trainium2 NeuronCores.

Reference semantics (per 2x2 window, stride 2, NHWC):
    r  = relu(x)
    s  = sum(r); ss = sum(r*r)
    out = ss / s   if s > 0 else 0

Sharding: pure data parallel over batch (64 -> 8 per core). Each core
processes x_local [8, 64, 64, 256] -> y_local [8, 32, 32, 256].

Pipeline (v4): partitions carry (bb, h) = 2 batches x 64 input rows.
  ACT:  R = relu(x) -> bf16
  DVE:  w-pair adds in bf16 (2x packed mode); Q = R*R on 2/3 of chunks
  PE :  h-pair reduction as matmul with a 0/1 halving matrix
        W[k, m] = 1 iff m == k//2; two groups pack into one PSUM tile
        ([0:64) from group A via W_A, [64:128) from group B via W_B);
        a third K=1 matmul accumulates eps into s so 0-windows give 0/eps
  DVE:  t = 1/s (custom recip), out = ss * t, both reading PSUM f32
"""

import sys

if "/opt/trn_rl_repo" not in sys.path:
    sys.path.insert(0, "/opt/trn_rl_repo")

import numpy as np

_B, _H, _W, _C = 8, 64, 64, 256  # per-core shard
_HO, _WO = _H // 2, _W // 2
_NP = 128                         # SBUF partitions
_F = 4096                         # floats per row chunk (16 w * 256 c)
_FH = _F // 2                     # w-pair reduced width
_PC = 512                         # psum piece width (f32, 1 bank)
_NG = 4                           # groups of (bb=2, h=64) partition rows
_EPS = 1e-30

_CACHE = {}


def _pin_act_table(bacc, mybir):
    """Route every activation to natural_log_exp_and_others so the kernel
    needs exactly one ACT function-table load. The compiler's greedy set
    choice otherwise alternates sets (~2.7us reload each)."""
    if getattr(bacc, "_arp_act_pin", False):
        return
    AF = mybir.ActivationFunctionType
    pin = {AF.Relu, AF.Square, AF.Ln, AF.Exp}
    orig = bacc.get_activation_tables

    def pinned(arch):
        tabs = orig(arch)
        keep = {f for f in pin if f in tabs.get("natural_log_exp_and_others", set())}
        return {
            name: (fns if name == "natural_log_exp_and_others" else fns - keep)
            for name, fns in tabs.items()
        }

    bacc.get_activation_tables = pinned
    bacc._arp_act_pin = True


def _build_nc():
    import concourse.bacc as bacc
    import concourse.tile as tile
    from concourse import mybir

    _pin_act_table(bacc, mybir)
    nc = bacc.Bacc("TRN2", target_bir_lowering=False, debug=False, num_devices=8)
    f32 = mybir.dt.float32
    bf16 = mybir.dt.bfloat16
    i32 = mybir.dt.int32
    x = nc.dram_tensor("x", [_B, _H, _W, _C], f32, kind="ExternalInput")
    y = nc.dram_tensor("y", [_B, _HO, _WO, _C], f32, kind="ExternalOutput")

    # [4, 128, 16384]: xg[bg, (bb, h), (w, c)]
    xg = x.ap().rearrange("(bg bb) h w c -> bg (bb h) (w c)", bb=2)
    # [2, 128, 8192]: yo[pair, (half, bb, ho), (wo, c)]
    yo = y.ap().rearrange("(pr hf bb) ho w c -> pr (hf bb ho) (w c)", pr=2, hf=2)

    relu = mybir.ActivationFunctionType.Relu
    square = mybir.ActivationFunctionType.Square
    is_ge = mybir.AluOpType.is_ge
    is_lt = mybir.AluOpType.is_lt

    with tile.TileContext(nc) as tc:
        with (
            tc.tile_pool(name="io", bufs=4) as io,
            tc.tile_pool(name="rq", bufs=8) as rq,
            tc.tile_pool(name="tmp", bufs=4) as tmp,
            tc.tile_pool(name="ot", bufs=3) as ot,
            tc.tile_pool(name="wt", bufs=1) as wt,
            tc.psum_pool(name="ps", bufs=2) as ps,
        ):
            # --- one-time: build the two halving matrices in SBUF ---
            # W_A[k, j] = 1 iff j == k//2       (cols 64.. are zero)
            # W_B[k, j] = 1 iff j == 64 + k//2  (cols ..64 are zero)
            WA = wt.tile([_NP, _NP], bf16, tag="WA")
            WB = wt.tile([_NP, _NP], bf16, tag="WB")
            d = wt.tile([_NP, _NP], i32, tag="d")
            ge = wt.tile([_NP, _NP], i32, tag="ge")
            lt = wt.tile([_NP, _NP], i32, tag="lt")
            wi = wt.tile([_NP, _NP], i32, tag="wi")
            for W, base in ((WA, 0), (WB, 128)):
                # d[p, j] = base + p - 2j; W = (d >= 0) & (d < 2)
                nc.gpsimd.iota(d[:], [[-2, _NP]], base=base, channel_multiplier=1)
                nc.vector.tensor_scalar(ge[:], d[:], 0, None, op0=is_ge)
                nc.vector.tensor_scalar(lt[:], d[:], 2, None, op0=is_lt)
                nc.vector.tensor_mul(wi[:], ge[:], lt[:])
                nc.vector.tensor_copy(W[:], wi[:])

            # warm the ACT table load + DVE recip custom-op path
            warm0 = wt.tile([_NP, 8], f32, tag="warm0")
            warm1 = wt.tile([_NP, 8], f32, tag="warm1")
            warmb = wt.tile([_NP, 8], bf16, tag="warmb")
            nc.vector.memset(warm0[:], 1.0)
            nc.scalar.activation(warmb[:], warm0[:], relu)
            nc.scalar.activation(warmb[:], warmb[:], square)
            nc.vector.reciprocal_approx_fast(warm1[:], warm0[:])

            # eps injectors: ones[1,128].T @ epsrow[1,N] accumulates eps
            # into every element of an s psum tile (runs on the idle PE)
            WE = wt.tile([1, _NP], bf16, tag="WE")
            epsr = wt.tile([1, _PC], bf16, tag="epsr")
            nc.vector.memset(WE[:], 1.0)
            nc.vector.memset(epsr[:], _EPS)

            def front(bg, c0, F, act_square):
                """Load + relu + square + w-pair adds for one group chunk.
                Returns (sw, ssw) bf16 [128, F/2] tiles."""
                EO = io.tile([_NP, F], f32, tag="EO")
                nc.sync.dma_start(EO[:], xg[bg, :, c0:c0 + F])
                R = rq.tile([_NP, F], bf16, tag="RQ")
                Q = rq.tile([_NP, F], bf16, tag="RQ")
                sw = tmp.tile([_NP, F // 2], bf16, tag="sw")
                ssw = tmp.tile([_NP, F // 2], bf16, tag="ssw")

                def wpair(t_):
                    v = t_[:].rearrange("p (w par c) -> p w par c", par=2, c=_C)
                    return v[:, :, 0, :], v[:, :, 1, :]

                def whalf(t_):
                    return t_[:].rearrange("p (w c) -> p w c", c=_C)

                nc.scalar.activation(R[:], EO[:], relu)
                Re, Ro = wpair(R)
                nc.vector.tensor_add(whalf(sw), Re, Ro)
                if act_square:
                    nc.scalar.activation(Q[:], R[:], square)
                else:
                    nc.vector.tensor_mul(Q[:], R[:], R[:])
                Qe, Qo = wpair(Q)
                nc.vector.tensor_add(whalf(ssw), Qe, Qo)
                return sw, ssw

            # schedule: pairs of groups; within a pair, chunk columns.
            # Each psum tile spans 2 banks; each matmul targets one bank
            # (ISA limit) but recip/mul then run at [128, 1024] width.
            # The first pair's first column is split fine-grained so all
            # engines ramp up quickly.
            def emit_pair(pair, c0, F, act_sq):
                bgA, bgB = 2 * pair, 2 * pair + 1
                FH2 = F // 2
                swA, sswA = front(bgA, c0, F, act_sq)
                swB, sswB = front(bgB, c0, F, act_sq)
                o = ot.tile([_NP, FH2], f32, tag="o")
                for pc in range(0, FH2, 2 * _PC):
                    pw = min(2 * _PC, FH2 - pc)
                    s_ps = ps.tile([_NP, pw], f32, tag="s")
                    q_ps = ps.tile([_NP, pw], f32, tag="q")
                    for h0 in range(0, pw, _PC):
                        sb = s_ps[:, h0:h0 + _PC]
                        qb = q_ps[:, h0:h0 + _PC]
                        sl = slice(pc + h0, pc + h0 + _PC)
                        nc.tensor.matmul(sb, WA[:], swA[:, sl],
                                         start=True, stop=False)
                        nc.tensor.matmul(sb, WB[:], swB[:, sl],
                                         start=False, stop=False)
                        nc.tensor.matmul(sb, WE[:], epsr[:],
                                         start=False, stop=True)
                        nc.tensor.matmul(qb, WA[:], sswA[:, sl],
                                         start=True, stop=False)
                        nc.tensor.matmul(qb, WB[:], sswB[:, sl],
                                         start=False, stop=True)
                    t = tmp.tile([_NP, pw], f32, tag="t")
                    nc.vector.reciprocal_approx_fast(t[:], s_ps[:])
                    nc.vector.tensor_mul(o[:, pc:pc + pw], q_ps[:], t[:])
                # stores go out on the ACT HWDGE queue so their sem
                # waits don't head-of-line block loads on the sync queue
                nc.scalar.dma_start(
                    yo[pair, :, c0 // 2:c0 // 2 + FH2], o[:]
                )

            sq_idx = 0
            for pair in range(2):
                plan = [(0, 1024), (1024, 1024), (2048, 2048)] if pair == 0 else []
                start = 4096 if pair == 0 else 0
                plan += [(c, _F) for c in range(start, _W * _C, _F)]
                for c0, F in plan:
                    emit_pair(pair, c0, F, sq_idx % 3 == 2)
                    sq_idx += 1

    nc.compile()
    return nc


def _get_nc():
    if "nc" not in _CACHE:
        _CACHE["nc"] = _build_nc()
    return _CACHE["nc"]


def kernel(x: np.ndarray) -> np.ndarray:
    from concourse.bass_utils import run_bass_kernel_spmd

    nc = _get_nc()
    x = np.ascontiguousarray(np.asarray(x, dtype=np.float32))
    shards = np.split(x, 8, axis=0)
    in_maps = [{"x": s} for s in shards]
    res = run_bass_kernel_spmd(nc, in_maps, list(range(8)))
    return np.concatenate([res.results[i]["y"] for i in range(8)], axis=0)


# revision 18
# speedup vs baseline: 1.0679x; 1.0679x over previous
"""AcceptRejectPooling2D on 8 Trainium2 NeuronCores.

Reference semantics (per 2x2 window, stride 2, NHWC):
    r  = relu(x)
    s  = sum(r); ss = sum(r*r)
    out = ss / s   if s > 0 else 0

Sharding: pure data parallel over batch (64 -> 8 per core). Each core
processes x_local [8, 64, 64, 256] -> y_local [8, 32, 32, 256].

Pipeline (v4): partitions carry (bb, h) = 2 batches x 64 input rows.
  ACT:  R = relu(x) -> bf16
  DVE:  w-pair adds in bf16 (2x packed mode); Q = R*R on 2/3 of chunks
  PE :  h-pair reduction as matmul with a 0/1 halving matrix
        W[k, m] = 1 iff m == k//2; two groups pack into one PSUM tile
        ([0:64) from group A via W_A, [64:128) from group B via W_B);
        a third K=1 matmul accumulates eps into s so 0-windows give 0/eps
  DVE:  t = 1/s (custom recip), out = ss * t, both reading PSUM f32
"""

import sys

if "/opt/trn_rl_repo" not in sys.path:
    sys.path.insert(0, "/opt/trn_rl_repo")

import numpy as np

_B, _H, _W, _C = 8, 64, 64, 256  # per-core shard
_HO, _WO = _H // 2, _W // 2
_NP = 128                         # SBUF partitions
_F = 4096                         # floats per row chunk (16 w * 256 c)
_FH = _F // 2                     # w-pair reduced width
_PC = 512                         # psum piece width (f32, 1 bank)
_NG = 4                           # groups of (bb=2, h=64) partition rows
_EPS = 1e-30

_CACHE = {}


def _pin_act_table(bacc, mybir):
    """Route every activation to natural_log_exp_and_others so the kernel
    needs exactly one ACT function-table load. The compiler's greedy set
    choice otherwise alternates sets (~2.7us reload each)."""
    if getattr(bacc, "_arp_act_pin", False):
        return
    AF = mybir.ActivationFunctionType
    pin = {AF.Relu, AF.Square, AF.Ln, AF.Exp}
    orig = bacc.get_activation_tables

    def pinned(arch):
        tabs = orig(arch)
        keep = {f for f in pin if f in tabs.get("natural_log_exp_and_others", set())}
        return {
            name: (fns if name == "natural_log_exp_and_others" else fns - keep)
            for name, fns in tabs.items()
        }

    bacc.get_activation_tables = pinned
    bacc._arp_act_pin = True


def _build_nc():
    import concourse.bacc as bacc
    import concourse.tile as tile
    from concourse import mybir

    _pin_act_table(bacc, mybir)
    nc = bacc.Bacc("TRN2", target_bir_lowering=False, debug=False, num_devices=8)
    f32 = mybir.dt.float32
    bf16 = mybir.dt.bfloat16
    i32 = mybir.dt.int32
    x = nc.dram_tensor("x", [_B, _H, _W, _C], f32, kind="ExternalInput")
    y = nc.dram_tensor("y", [_B, _HO, _WO, _C], f32, kind="ExternalOutput")

    # [4, 128, 16384]: xg[bg, (bb, h), (w, c)]
    xg = x.ap().rearrange("(bg bb) h w c -> bg (bb h) (w c)", bb=2)
    # [2, 128, 8192]: yo[pair, (half, bb, ho), (wo, c)]
    yo = y.ap().rearrange("(pr hf bb) ho w c -> pr (hf bb ho) (w c)", pr=2, hf=2)

    relu = mybir.ActivationFunctionType.Relu
    square = mybir.ActivationFunctionType.Square
    is_ge = mybir.AluOpType.is_ge
    is_lt = mybir.AluOpType.is_lt

    with tile.TileContext(nc) as tc:
        with (
            tc.tile_pool(name="io", bufs=4) as io,
            tc.tile_pool(name="rq", bufs=8) as rq,
            tc.tile_pool(name="tmp", bufs=4) as tmp,
            tc.tile_pool(name="ot", bufs=3) as ot,
            tc.tile_pool(name="wt", bufs=1) as wt,
            tc.psum_pool(name="ps", bufs=4) as ps,
        ):
            # --- one-time: build the two halving matrices in SBUF ---
            # W_A[k, j] = 1 iff j == k//2       (cols 64.. are zero)
            # W_B[k, j] = 1 iff j == 64 + k//2  (cols ..64 are zero)
            WA = wt.tile([_NP, _NP], bf16, tag="WA")
            WB = wt.tile([_NP, _NP], bf16, tag="WB")
            d = wt.tile([_NP, _NP], i32, tag="d")
            ge = wt.tile([_NP, _NP], i32, tag="ge")
            lt = wt.tile([_NP, _NP], i32, tag="lt")
            wi = wt.tile([_NP, _NP], i32, tag="wi")
            for W, base in ((WA, 0), (WB, 128)):
                # d[p, j] = base + p - 2j; W = (d >= 0) & (d < 2)
                nc.gpsimd.iota(d[:], [[-2, _NP]], base=base, channel_multiplier=1)
                nc.vector.tensor_scalar(ge[:], d[:], 0, None, op0=is_ge)
                nc.vector.tensor_scalar(lt[:], d[:], 2, None, op0=is_lt)
                nc.vector.tensor_mul(wi[:], ge[:], lt[:])
                nc.vector.tensor_copy(W[:], wi[:])

            # warm the ACT table load + DVE recip custom-op path
            warm0 = wt.tile([_NP, 8], f32, tag="warm0")
            warm1 = wt.tile([_NP, 8], f32, tag="warm1")
            warmb = wt.tile([_NP, 8], bf16, tag="warmb")
            nc.vector.memset(warm0[:], 1.0)
            nc.scalar.activation(warmb[:], warm0[:], relu)
            nc.scalar.activation(warmb[:], warmb[:], square)
            nc.vector.reciprocal_approx_fast(warm1[:], warm0[:])

            # eps injectors: ones[1,128].T @ epsrow[1,N] accumulates eps
            # into every element of an s psum tile (runs on the idle PE)
            WE = wt.tile([1, _NP], bf16, tag="WE")
            epsr = wt.tile([1, _PC], bf16, tag="epsr")
            nc.vector.memset(WE[:], 1.0)
            nc.vector.memset(epsr[:], _EPS)

            def front(bg, c0, F, act_square):
                """Load + relu + square + w-pair adds for one group chunk.
                Returns (sw, ssw) bf16 [128, F/2] tiles."""
                EO = io.tile([_NP, F], f32, tag="EO")
                nc.sync.dma_start(EO[:], xg[bg, :, c0:c0 + F])
                R = rq.tile([_NP, F], bf16, tag="RQ")
                Q = rq.tile([_NP, F], bf16, tag="RQ")
                sw = tmp.tile([_NP, F // 2], bf16, tag="sw")
                ssw = tmp.tile([_NP, F // 2], bf16, tag="ssw")

                def wpair(t_):
                    v = t_[:].rearrange("p (w par c) -> p w par c", par=2, c=_C)
                    return v[:, :, 0, :], v[:, :, 1, :]

                def whalf(t_):
                    return t_[:].rearrange("p (w c) -> p w c", c=_C)

                nc.scalar.activation(R[:], EO[:], relu)
                Re, Ro = wpair(R)
                nc.vector.tensor_add(whalf(sw), Re, Ro)
                if act_square:
                    nc.scalar.activation(Q[:], R[:], square)
                else:
                    nc.vector.tensor_mul(Q[:], R[:], R[:])
                Qe, Qo = wpair(Q)
                nc.vector.tensor_add(whalf(ssw), Qe, Qo)
                return sw, ssw

            # schedule: pairs of groups; within a pair, chunk columns.
            # Each psum tile spans 2 banks; each matmul targets one bank
            # (ISA limit) but recip/mul then run at [128, 1024] width.
            # The first pair's first column is split fine-grained so all
            # engines ramp up quickly.
            def emit_pair(pair, c0, F, act_sq):
                bgA, bgB = 2 * pair, 2 * pair + 1
                FH2 = F // 2
                swA, sswA = front(bgA, c0, F, act_sq)
                swB, sswB = front(bgB, c0, F, act_sq)
                o = ot.tile([_NP, FH2], f32, tag="o")
                for pc in range(0, FH2, _PC):
                    pw = min(_PC, FH2 - pc)
                    s_ps = ps.tile([_NP, pw], f32, tag="s")
                    q_ps = ps.tile([_NP, pw], f32, tag="q")
                    sl = slice(pc, pc + pw)
                    nc.tensor.matmul(s_ps[:], WA[:], swA[:, sl],
                                     start=True, stop=False)
                    nc.tensor.matmul(s_ps[:], WB[:], swB[:, sl],
                                     start=False, stop=False)
                    nc.tensor.matmul(s_ps[:], WE[:], epsr[:, :pw],
                                     start=False, stop=True)
                    nc.tensor.matmul(q_ps[:], WA[:], sswA[:, sl],
                                     start=True, stop=False)
                    nc.tensor.matmul(q_ps[:], WB[:], sswB[:, sl],
                                     start=False, stop=True)
                    t = tmp.tile([_NP, pw], f32, tag="t")
                    nc.vector.reciprocal_approx_fast(t[:], s_ps[:])
                    nc.vector.tensor_mul(o[:, pc:pc + pw], q_ps[:], t[:])
                # stores go out on the ACT HWDGE queue so their sem
                # waits don't head-of-line block loads on the sync queue
                nc.scalar.dma_start(
                    yo[pair, :, c0 // 2:c0 // 2 + FH2], o[:]
                )

            sq_idx = 0
            for pair in range(2):
                plan = [(0, 1024), (1024, 1024), (2048, 2048)] if pair == 0 else []
                start = 4096 if pair == 0 else 0
                plan += [(c, _F) for c in range(start, _W * _C, _F)]
                for c0, F in plan:
                    emit_pair(pair, c0, F, sq_idx % 3 == 2)
                    sq_idx += 1

    nc.compile()
    return nc


def _get_nc():
    if "nc" not in _CACHE:
        _CACHE["nc"] = _build_nc()
    return _CACHE["nc"]


def kernel(x: np.ndarray) -> np.ndarray:
    from concourse.bass_utils import run_bass_kernel_spmd

    nc = _get_nc()
    x = np.ascontiguousarray(np.asarray(x, dtype=np.float32))
    shards = np.split(x, 8, axis=0)
    in_maps = [{"x": s} for s in shards]
    res = run_bass_kernel_spmd(nc, in_maps, list(range(8)))
    return np.concatenate([res.results[i]["y"] for i in range(8)], axis=0)


# revision 19
# speedup vs baseline: 1.1189x; 1.0478x over previous
"""AcceptRejectPooling2D on 8 Trainium2 NeuronCores.

Reference semantics (per 2x2 window, stride 2, NHWC):
    r  = relu(x)
    s  = sum(r); ss = sum(r*r)
    out = ss / s   if s > 0 else 0

Sharding: pure data parallel over batch (64 -> 8 per core). Each core
processes x_local [8, 64, 64, 256] -> y_local [8, 32, 32, 256].

Layout per core: rows (b, h) of length W*C = 16384 floats. Output row
p = (b, ho) needs input rows 2p (even h) and 2p+1 (odd h). 256 output
rows = 2 partition groups of 128. Row chunks of F floats stream through
SBUF; within a chunk the w-pair reduction is a strided tensor_add.

bf16 intermediate pipeline (rel-err budget 2e-2 allows it): ACT casts
relu(x) f32->bf16; squares and the w-pair adds run on bf16 operands so
the DVE's 2x packed mode applies. s and ss promote to f32 at their
h-combine; out = ss / s is a single DVE tensor_tensor divide.
"""

import sys

if "/opt/trn_rl_repo" not in sys.path:
    sys.path.insert(0, "/opt/trn_rl_repo")

import numpy as np

_B, _H, _W, _C = 8, 64, 64, 256  # per-core shard
_HO, _WO = _H // 2, _W // 2
_NP = 128                         # SBUF partitions
_F = 2048                         # floats per row chunk (8 w * 256 c)
_FO = _F // 2
_NG = (_B * _HO) // _NP           # partition groups (2)
_EPS = 1e-30

_CACHE = {}


def _pin_act_table(bacc, mybir):
    """Route every activation to natural_log_exp_and_others (which holds
    Relu, Square, Ln AND Exp) so the kernel needs exactly one ACT
    function-table load. The compiler's per-instruction greedy set choice
    otherwise alternates sets (~2.7us reload each). Only the in-memory
    choice list is edited; set ids / loaded table bytes are unchanged.
    """
    if getattr(bacc, "_arp_act_pin", False):
        return
    AF = mybir.ActivationFunctionType
    pin = {AF.Relu, AF.Square, AF.Ln, AF.Exp}
    orig = bacc.get_activation_tables

    def pinned(arch):
        return {
            name: (fns if name == "natural_log_exp_and_others" else fns - pin)
            for name, fns in orig(arch).items()
        }

    bacc.get_activation_tables = pinned
    bacc._arp_act_pin = True


def _build_nc():
    import concourse.bacc as bacc
    import concourse.tile as tile
    from concourse import mybir

    _pin_act_table(bacc, mybir)
    nc = bacc.Bacc("TRN2", target_bir_lowering=False, debug=False, num_devices=8)
    f32 = mybir.dt.float32
    bf16 = mybir.dt.bfloat16
    x = nc.dram_tensor("x", [_B, _H, _W, _C], f32, kind="ExternalInput")
    y = nc.dram_tensor("y", [_B, _HO, _WO, _C], f32, kind="ExternalOutput")

    # [256, 2, 16384]: xv[(b, ho), par, (w, c)] with par = h % 2
    xv = x.ap().rearrange("b (hh par) w c -> (b hh) par (w c)", par=2)
    # [256, 8192]
    yv = y.ap().rearrange("b i j c -> (b i) (j c)")

    relu = mybir.ActivationFunctionType.Relu
    square = mybir.ActivationFunctionType.Square
    add = mybir.AluOpType.add

    with tile.TileContext(nc) as tc:
        with (
            tc.tile_pool(name="io", bufs=4) as io,
            tc.tile_pool(name="rq", bufs=4) as rq,
            tc.tile_pool(name="tmp", bufs=3) as tmp,
            tc.tile_pool(name="ot", bufs=3) as ot,
        ):
            def emit(g, c0, F, act_square, o_tile, o_off, o_w):
                # One iteration covers both h-rows (E|O fused along free dim)
                # of 128 output rows x F floats of row. Output lands in
                # o_tile[:, o_off:o_off+F/2]; caller stores when full.
                FO = F // 2
                p0, p1 = g * _NP, (g + 1) * _NP
                EO = io.tile([_NP, 2 * F], f32, tag="EO")
                eov = EO[:].rearrange("p (par f) -> p par f", par=2)
                nc.sync.dma_start(eov, xv[p0:p1, :, c0:c0 + F])

                R = rq.tile([_NP, 2 * F], bf16, tag="RQ")
                Q = rq.tile([_NP, 2 * F], bf16, tag="RQ")
                sw = tmp.tile([_NP, F], bf16, tag="sw")
                ssw = tmp.tile([_NP, F], bf16, tag="ssw")
                s = tmp.tile([_NP, FO], f32, tag="s")
                ss = tmp.tile([_NP, FO], f32, tag="ss")
                t = tmp.tile([_NP, FO], f32, tag="t")

                def pairs(tile_):
                    # [128, 2F] -> even/odd w views [128, 2, F//512, 256]
                    v = tile_[:].rearrange(
                        "p (h w par c) -> p h w par c", h=2, par=2, c=_C
                    )
                    return v[:, :, :, 0, :], v[:, :, :, 1, :]

                def halfpair(tile_):
                    return tile_[:].rearrange("p (h w c) -> p h w c", h=2, c=_C)

                # relu + downcast to bf16 in one ACT pass
                nc.scalar.activation(R[:], EO[:], relu)
                Re, Ro = pairs(R)
                # w-pair adds for both h-rows in one bf16 op: sw = [sE | sO]
                nc.vector.tensor_add(halfpair(sw), Re, Ro)
                # s = (sE + eps) + sO in f32; eps guards 0/0 zero windows
                nc.vector.scalar_tensor_tensor(
                    s[:], sw[:, :FO], _EPS, sw[:, FO:], op0=add, op1=add
                )
                nc.vector.reciprocal_approx_fast(t[:], s[:])

                if act_square:
                    nc.scalar.activation(Q[:], R[:], square)
                else:
                    nc.vector.tensor_mul(Q[:], R[:], R[:])
                Qe, Qo = pairs(Q)
                nc.vector.tensor_add(halfpair(ssw), Qe, Qo)
                nc.vector.tensor_add(ss[:], ssw[:, :FO], ssw[:, FO:])
                nc.vector.tensor_mul(o_tile[:, o_off:o_off + FO], ss[:], t[:])
                if o_off + FO == o_w:
                    nc.sync.dma_start(
                        yv[p0:p1, (c0 + F) // 2 - o_w:(c0 + F) // 2],
                        o_tile[:, :o_w],
                    )

            # Warm the ACT function-table (~2.7us load) and the DVE custom
            # recip path on dummy data so they overlap the first input DMA
            # instead of delaying the first real relu.
            warm0 = tmp.tile([_NP, 8], f32, tag="warm0")
            warm1 = tmp.tile([_NP, 8], f32, tag="warm1")
            warmb = tmp.tile([_NP, 8], bf16, tag="warmb")
            nc.vector.memset(warm0[:], 1.0)
            nc.scalar.activation(warmb[:], warm0[:], relu)
            nc.scalar.activation(warmb[:], warmb[:], square)
            nc.vector.reciprocal_approx_fast(warm1[:], warm0[:])

            row = _W * _C
            full_idx = 0
            for g in range(_NG):
                c = 0
                if g == 0:
                    # fine-grained warmup chunks so compute starts early;
                    # each stores its own (small) output immediately
                    for fw in (512, 512, 1024):
                        o_t = ot.tile([_NP, fw // 2], f32, tag="o")
                        emit(g, c, fw, False, o_t, 0, fw // 2)
                        c += fw
                # fine-grained cooldown chunks on the last group shrink the
                # (last compute -> last store) tail
                tail = (1024, 512, 512) if g == _NG - 1 else ()
                stop = row - sum(tail)
                # full chunks: pair outputs into 1 MiB stores; squares go to
                # ACT on 7 of 8 chunks to balance ACT vs DVE (DVE keeps the
                # recip+mul division tail)
                pend, pend_off = None, 0
                while c < stop:
                    if pend is None:
                        nfull = (stop - c) // _F
                        o_w = _FO * (2 if nfull >= 2 else 1)
                        pend = ot.tile([_NP, o_w], f32, tag="o")
                        pend_off, pend_w = 0, o_w
                    emit(g, c, _F, full_idx % 8 != 0, pend, pend_off, pend_w)
                    pend_off += _FO
                    if pend_off == pend_w:
                        pend = None
                    full_idx += 1
                    c += _F
                for fw in tail:
                    # cooldown squares on ACT: the DVE tail (divide) is the
                    # critical path at the end while ACT has slack
                    o_t = ot.tile([_NP, fw // 2], f32, tag="o")
                    emit(g, c, fw, True, o_t, 0, fw // 2)
                    c += fw

    nc.compile()
    return nc


def _get_nc():
    if "nc" not in _CACHE:
        _CACHE["nc"] = _build_nc()
    return _CACHE["nc"]


def kernel(x: np.ndarray) -> np.ndarray:
    from concourse.bass_utils import run_bass_kernel_spmd

    nc = _get_nc()
    x = np.ascontiguousarray(np.asarray(x, dtype=np.float32))
    shards = np.split(x, 8, axis=0)
    in_maps = [{"x": s} for s in shards]
    res = run_bass_kernel_spmd(nc, in_maps, list(range(8)))
    return np.concatenate([res.results[i]["y"] for i in range(8)], axis=0)
